# revision 29
# baseline (speedup 1.0000x reference)
"""Trainium2 Bass kernel for nn_BinaryQuantumClassifier.

Math: the 4-qubit circuit collapses to a closed form. Per sample, with
theta_j = pi * (x @ W_ctq.T + b_ctq)_j  (j = 4r + i, reuse r, qubit i):
    d_i(theta) = a_i + b_i sin(theta) + c_i cos(theta)
              = a_i + R_i sin(pi * (y + b_ctq_j + phi_i/pi))
(R = hypot(b, c), phi = atan2(c, b)), and the CNOT chain maps Z-expectations
to products of the d_i:
    z0 = d1 d2 d3, z1 = d0 d1, z2 = d0 d1 d2, z3 = d0 d1 d2 d3.
Output = (mean over r of z) @ W_cls.T + b_cls.

Device plan per core (8192 samples). The kernel is HBM-bound on streaming x
(~360-420 GB/s/core aggregate, reachable only with BOTH HWDGE queues), so x
is sent as fp16 (2 B/elem; rel err ~2.4e-3, gate 2e-2) in 16 tiles of 4
sample-groups, alternating between the SP and Activation HWDGE queues, all
triggers up front. x is the PE's STATIONARY operand:
  lhsT = x-chunk [128 D x 128 samples], rhs = W-chunk [128 D x 8] (fp16),
  out[128 samples, 8] accumulated straight into a per-chunk PSUM tile E
  (free col = 8*g + j, j-minor) — no PSUM->SBUF assembly copy.
The phase shift bs_j = b_ctq_j + phi/pi is added in PSUM by one extra
K=2 matmul per chunk (ones lhsT, fp16 hi/lo bias rows => fp32-exact).
All constants ride ONE fp16 misc tile loaded first on the sync queue;
0.5*W_cls is a [128, 8*24] block read through per-chunk strided views.
Sample groups form geometrically shrinking chunks (24/20/12/8 groups) so
each chunk's epilogue overlaps later chunks' DMA+matmul and the tail chunk
is small:
  k2 = ((E + 1.5*2^24) - 1.5*2^24) rounds to the nearest even integer
  (exact range reduction), r = E - k2 in [-1, 1], ScalarE Sin once,
  d = Rw*s + aw, CNOT products via stride-4 views (both reuse halves per
  op; z0/z3 on GpSimd), then per class: Zw = Z * WC (strided const view),
  segmented tensor_reduce over the 8 (k, r) slots per group, + b_cls.
Per-chunk output store on the sync queue.
"""

import numpy as np

import concourse.bass as bass
import concourse.mybir as mybir
from concourse import bass_utils
from concourse.tile import TileContext

B, D, NQ = 65536, 512, 4
NCORES = 8
BC = B // NCORES            # 8192 samples per core
NCH = D // 128              # 4 K-chunks
NG = BC // 128              # 64 sample-groups per core (128 samples each)
GW = NCH * 128              # 512: x columns per sample-group
TGROUPS = [4] * 16                                # groups per x-tile
CHUNKS = [(0, 7, 28), (7, 6, 24), (13, 3, 12)]
# x-tile -> DMA queue engine. Only the two HWDGE queues (SP=0, Act=1) reach
# the ~360 GB/s aggregate HBM share; adding the Pool SWDGE queue consistently
# REDUCED aggregate bandwidth, so x rides sync/scalar alternating.
TQUEUE = [0, 1] * 8
M2 = float(np.float32(1.5 * 2 ** 24))   # round-to-even-integer magic
PI = float(np.pi)
MM_DT = mybir.dt.float16    # PE operand / const dtype
F32 = mybir.dt.float32
AL = mybir.AluOpType
AF = mybir.ActivationFunctionType
AX = mybir.AxisListType
# misc1 (sync queue): wfa 32 | ones 128 | bias 224
MW_WFA, MW_ONES, MW_BIAS, MW1 = 0, 32, 160, 384
# misc2 (scalar queue): Rw 224 | aw 224 | per-chunk 2*8w WC blocks
MW_RW, MW_AW, MW_WC = 0, 224, 448
_WCOFF = [MW_WC]
for (_, _, _w) in CHUNKS:
    _WCOFF.append(_WCOFF[-1] + 2 * 8 * _w)
MW2 = _WCOFF[-1] + (-_WCOFF[-1]) % 64          # pad to 64


def _split_waits(nc, max_waits=1):
    """walrus in this env accepts at most one sync-wait per instruction;
    move extras onto preceding same-engine NoOps."""
    for fn in nc.m.functions:
        for blk in fn.blocks:
            new_list = []
            for inst in blk.instructions:
                si = inst.sync_info
                if si is not None and len(si.on_wait) > max_waits:
                    waits = list(si.on_wait)
                    keep, extra = waits[-max_waits:], waits[:-max_waits]
                    for k, w in enumerate(extra):
                        new_list.append(mybir.InstNoOp(
                            name=f"{inst.name}-ws{k}", engine=inst.engine,
                            ins=[], outs=[],
                            sync_info=mybir.SyncInfo(on_wait=[w], on_update=[])))
                    si.on_wait = keep
                    inst.sync_info = si
                new_list.append(inst)
            blk.instructions = new_list


def _build_nc(bc2):
    """bc2: [2] = b_cls immediates (everything else rides the misc tile)."""
    nc = bass.Bass("TRN2", target_bir_lowering=False)
    # x relayout: xa[p, g*512 + k*128 + ms] = x_core[128 g + ms, 128 k + p]
    xa_d = nc.dram_tensor("xa", [128, BC * NCH], MM_DT, kind="ExternalInput").ap()
    misc1_d = nc.dram_tensor("misc1", [128, MW1], MM_DT, kind="ExternalInput").ap()
    misc2_d = nc.dram_tensor("misc2", [128, MW2], MM_DT, kind="ExternalInput").ap()
    o_d = nc.dram_tensor("o", [128, 2 * NG], F32, kind="ExternalOutput").ap()

    tstart = np.cumsum([0] + TGROUPS)

    with TileContext(nc) as tc:
        with tc.tile_pool(name="wp", bufs=1) as wpool, \
             tc.tile_pool(name="xp", bufs=1) as xpool, \
             tc.tile_pool(name="pp", bufs=1, space="PSUM") as pspool, \
             tc.tile_pool(name="ep", bufs=1) as epool:
            # --- all DMA triggers up front, one const tile per queue ---
            misc = wpool.tile([128, MW1], MM_DT, name="misc")
            nc.sync.dma_start(misc[:], misc1_d[:])
            misc2 = wpool.tile([128, MW2], MM_DT, name="misc2")
            nc.scalar.dma_start(misc2[:], misc2_d[:])
            ones = misc[0:2, MW_ONES:MW_ONES + 128]
            Rw = misc2[:, MW_RW:MW_RW + 224]
            aw = misc2[:, MW_AW:MW_AW + 224]

            engs = [nc.sync, nc.scalar, nc.gpsimd]
            xts = []
            for t, G in enumerate(TGROUPS):
                xt = xpool.tile([128, G * GW], MM_DT, tag=f"xt{t}", name=f"xt{t}")
                engs[TQUEUE[t]].dma_start(
                    xt[:], xa_d[:, tstart[t] * GW:tstart[t + 1] * GW])
                xts.append(xt)

            for ci, (t0, nt, w) in enumerate(CHUNKS):
                W = 8 * w             # chunk PSUM width, col = 8*g + j
                E = pspool.tile([128, W], F32, name=f"E{ci}")
                # phase shift first: E = bs_j (start=True zeroes the region)
                nc.tensor.matmul(E[:, 0:W], ones,
                                 misc[0:2, MW_BIAS:MW_BIAS + W],
                                 start=True, stop=False, skip_group_check=True)
                g0 = 0
                for t in range(t0, t0 + nt):
                    xt = xts[t]
                    for mm in range(TGROUPS[t]):
                        for k in range(NCH):
                            off = mm * GW + k * 128
                            col = 8 * (g0 + mm)
                            nc.tensor.matmul(E[:, col:col + 8],
                                             xt[:, off:off + 128],
                                             misc[:, MW_WFA + 8 * k:MW_WFA + 8 * k + 8],
                                             start=False, stop=(k == NCH - 1),
                                             skip_group_check=True)
                    g0 += TGROUPS[t]

                # ---- epilogue for this chunk ----
                vec, gp = nc.vector, nc.gpsimd
                k2 = epool.tile([128, W], F32, name=f"k2_{ci}")
                r_ = epool.tile([128, W], F32, name=f"r_{ci}")
                s_ = epool.tile([128, W], F32, name=f"s_{ci}")
                t1 = epool.tile([128, W], F32, name=f"t1_{ci}")
                d_ = epool.tile([128, W], F32, name=f"d_{ci}")
                vec.tensor_scalar(k2[:], E[:], M2, M2, AL.add, AL.subtract)
                vec.tensor_sub(r_[:], E[:], k2[:])           # E mod 2 -> [-1, 1]
                nc.scalar.activation(s_[:], r_[:], AF.Sin, scale=PI)
                vec.tensor_mul(t1[:], s_[:], Rw[:, 0:W])
                vec.tensor_add(d_[:], t1[:], aw[:, 0:W])

                # stride-4 views: d4[:, i, :] = d_i for both r, interleaved
                d4 = d_.rearrange("p (u q) -> p q u", q=4)

                def di(i):
                    return d4[:, i, :]                        # [128, 2w] @4

                # products for both reuse halves ([128, 2w], r-interleaved),
                # laid out in k-blocks: Z[:, 2w*k + 2g + r] = z_k^r(group g)
                u_ = epool.tile([128, 2 * w], F32, name=f"u_{ci}")
                Z_ = epool.tile([128, 8 * w], F32, name=f"Z_{ci}")

                def zk(k):
                    return Z_[:, 2 * w * k:2 * w * (k + 1)]

                vec.tensor_mul(u_[:], di(1), di(2))           # d1 d2
                gp.tensor_mul(zk(1), di(0), di(1))            # z1
                vec.tensor_mul(zk(2), di(0), u_[:])           # z2
                gp.tensor_mul(zk(0), u_[:], di(3))            # z0
                gp.tensor_mul(zk(3), zk(2), di(3))            # z3

                # final linear via weighted segmented reduce:
                # out_c(g) = sum_{k,r} 0.5*W_cls[c,k] * z_k^r(g) + b_cls[c]
                O2 = epool.tile([128, 2 * w], F32, name=f"O2_{ci}")
                Zw = epool.tile([128, 2, 8 * w], F32, name=f"Zw_{ci}")
                for c in range(2):
                    wcs = misc2[:, _WCOFF[ci] + c * 8 * w:
                                _WCOFF[ci] + (c + 1) * 8 * w]
                    (vec if c == 0 else gp).tensor_mul(Zw[:, c, :], Z_[:], wcs)
                    red = Zw[:, c, :].rearrange("p (k g r) -> p g k r",
                                                k=4, r=2)    # [p, w, 4, 2]
                    vec.tensor_reduce(O2[:, c * w:(c + 1) * w], red,
                                      AX.XY, AL.add)
                    vec.tensor_scalar_add(O2[:, c * w:(c + 1) * w],
                                          O2[:, c * w:(c + 1) * w], bc2[c])
                off = 2 * tstart[t0]
                nc.sync.dma_start(o_d[:, off:off + 2 * w], O2[:])

    return nc


_NC_CACHE = {}


def _get_nc(consts, split=True):
    (bc2,) = consts
    key = ("nc", split, bc2)
    if key not in _NC_CACHE:
        nc = _build_nc(bc2)
        if split:
            _split_waits(nc)
        _NC_CACHE[key] = nc
    return _NC_CACHE[key]


def _qubit_abc(q_params):
    """Exact (a_i, b_i, c_i) with d_i(theta) = a + b sin(theta) + c cos(theta)."""
    out = np.zeros((NQ, 3), np.float64)
    for i in range(NQ):
        pa, pb, pc = [float(v) for v in q_params[3 * i:3 * i + 3]]

        def rx(t):
            return np.array([[np.cos(t / 2), -1j * np.sin(t / 2)],
                             [-1j * np.sin(t / 2), np.cos(t / 2)]])

        def ry(t):
            return np.array([[np.cos(t / 2), -np.sin(t / 2)],
                             [np.sin(t / 2), np.cos(t / 2)]])

        def rz(t):
            return np.array([[np.exp(-0.5j * t), 0], [0, np.exp(0.5j * t)]])

        H = np.array([[1, 1], [1, -1]]) / np.sqrt(2)
        U = rz(pc) @ ry(pb) @ rx(pa)

        def dfun(theta):
            v = U @ ry(theta) @ H @ np.array([1.0, 0.0])
            pr = np.abs(v) ** 2
            return pr[0] - pr[1]

        d0, dpi, dh = dfun(0.0), dfun(np.pi), dfun(np.pi / 2)
        a = (d0 + dpi) / 2
        c = (d0 - dpi) / 2
        b = dh - a
        out[i] = (a, b, c)
    return out


def _make_consts(b_ctq, q_params, W_cls, b_cls):
    """b_cls immediates + the all-in-one misc const tile (fp16)."""
    abc = _qubit_abc(q_params)
    R8, a8, bs = np.zeros(8), np.zeros(8), np.zeros(8)
    for j in range(8):
        a, b, c_ = abc[j % 4]
        R8[j] = np.hypot(b, c_)
        a8[j] = a
        bs[j] = b_ctq[j] + np.arctan2(c_, b) / np.pi
    bc2 = tuple(float(np.float32(v)) for v in b_cls)

    misc = np.zeros((128, MW1), np.float16)
    misc[:, MW_ONES:MW_ONES + 128] = 1.0
    # bias rows: row0 = fp16 hi, row1 = residual lo (hi+lo == fp32 bs)
    bs_t = np.tile(bs, 224 // 8)
    bhi = bs_t.astype(np.float16)
    misc[0, MW_BIAS:MW_BIAS + 224] = bhi
    misc[1, MW_BIAS:MW_BIAS + 224] = (bs_t - bhi.astype(np.float64)).astype(np.float16)
    misc2 = np.zeros((128, MW2), np.float16)
    misc2[:, MW_RW:MW_RW + 224] = np.tile(R8, 224 // 8).astype(np.float16)
    misc2[:, MW_AW:MW_AW + 224] = np.tile(a8, 224 // 8).astype(np.float16)
    wp = 0.5 * np.asarray(W_cls, np.float64)      # mean over r folded in
    for ci, (_, _, w) in enumerate(CHUNKS):
        for c in range(2):
            for k in range(4):
                lo = _WCOFF[ci] + c * 8 * w + k * 2 * w
                misc2[:, lo:lo + 2 * w] = np.float16(wp[c, k])
    return (bc2,), misc, misc2


def make_in_maps(x, W_ctq, b_ctq, q_params, W_cls, b_cls):
    consts, misc, misc2 = _make_consts(np.asarray(b_ctq, np.float32),
                                       np.asarray(q_params, np.float32),
                                       np.asarray(W_cls, np.float32),
                                       np.asarray(b_cls, np.float32))
    wt = np.asarray(W_ctq, np.float32).T                        # [512, 8]
    misc[:, MW_WFA:MW_WFA + 32] = \
        wt.reshape(NCH, 128, 8).transpose(1, 0, 2).reshape(128, 32)
    misc = np.ascontiguousarray(misc)
    x = np.asarray(x, np.float32)
    in_maps = []
    for c in range(NCORES):
        xs = x[c * BC:(c + 1) * BC]                             # [8192, 512]
        # relayout: [p, g*512 + k*128 + ms] = xs[128 g + ms, 128 k + p]
        xa = np.ascontiguousarray(
            xs.reshape(NG, 128, NCH, 128).transpose(3, 0, 2, 1)
              .reshape(128, BC * NCH)).astype(np.float16)
        in_maps.append({"xa": xa, "misc1": misc, "misc2": misc2})
    return in_maps, consts


def assemble_output(results):
    tstart = np.cumsum([0] + TGROUPS)
    out = np.empty((B, 2), np.float32)
    for core in range(NCORES):
        o = results[core]["o"]                                   # [128, 2*NG]
        for (t0, nt, w) in CHUNKS:
            gb, off = tstart[t0], 2 * tstart[t0]
            for c in range(2):
                # o[p, off + c*w + g] = out_c(sample 128 (gb+g) + p)
                out[core * BC + 128 * gb:core * BC + 128 * (gb + w), c] = \
                    o[:, off + c * w:off + (c + 1) * w].T.reshape(-1)
    return out


def kernel(x, W_ctq, b_ctq, q_params, W_cls, b_cls):
    in_maps, consts = make_in_maps(x, W_ctq, b_ctq, q_params, W_cls, b_cls)
    nc = _get_nc(consts)
    res = bass_utils.run_bass_kernel_spmd(nc, in_maps, core_ids=list(range(NCORES)))
    return assemble_output(res.results)


# revision 33
# speedup vs baseline: 1.1770x; 1.1770x over previous
"""Trainium2 Bass kernel for nn_BinaryQuantumClassifier.

Math: the 4-qubit circuit collapses to a closed form. Per sample, with
theta_j = pi * (x @ W_ctq.T + b_ctq)_j  (j = 4r + i, reuse r, qubit i):
    d_i(theta) = a_i + b_i sin(theta) + c_i cos(theta)
              = a_i + R_i sin(pi * (y + b_ctq_j + phi_i/pi))
(R = hypot(b, c), phi = atan2(c, b)), and the CNOT chain maps Z-expectations
to products of the d_i:
    z0 = d1 d2 d3, z1 = d0 d1, z2 = d0 d1 d2, z3 = d0 d1 d2 d3.
Output = (mean over r of z) @ W_cls.T + b_cls.

Device plan per core (8192 samples). The kernel is HBM-bound on streaming x
(~360-420 GB/s/core aggregate, reachable only with BOTH HWDGE queues), so x
is sent as fp16 (2 B/elem; rel err ~2.4e-3, gate 2e-2) in 16 tiles of 4
sample-groups, alternating between the SP and Activation HWDGE queues, all
triggers up front. x is the PE's STATIONARY operand:
  lhsT = x-chunk [128 D x 128 samples], rhs = W-chunk [128 D x 8] (fp16),
  out[128 samples, 8] accumulated straight into a per-chunk PSUM tile E
  (free col = 8*g + j, j-minor) — no PSUM->SBUF assembly copy.
The phase shift bs_j = b_ctq_j + phi/pi is added in PSUM by one extra
K=2 matmul per chunk (ones lhsT, fp16 hi/lo bias rows => fp32-exact).
All constants ride ONE fp16 misc tile loaded first on the sync queue;
0.5*W_cls is a [128, 8*24] block read through per-chunk strided views.
Sample groups form geometrically shrinking chunks (24/20/12/8 groups) so
each chunk's epilogue overlaps later chunks' DMA+matmul and the tail chunk
is small:
  k2 = ((E + 1.5*2^24) - 1.5*2^24) rounds to the nearest even integer
  (exact range reduction), r = E - k2 in [-1, 1], ScalarE Sin once,
  d = Rw*s + aw, CNOT products via stride-4 views (both reuse halves per
  op; z0/z3 on GpSimd), then per class: Zw = Z * WC (strided const view),
  segmented tensor_reduce over the 8 (k, r) slots per group, + b_cls.
Per-chunk output store on the sync queue.
"""

import numpy as np

import concourse.bass as bass
import concourse.mybir as mybir
from concourse import bass_utils
from concourse.tile import TileContext

B, D, NQ = 65536, 512, 4
NCORES = 8
BC = B // NCORES            # 8192 samples per core
NCH = D // 128              # 4 K-chunks
NG = BC // 128              # 64 sample-groups per core (128 samples each)
GW = NCH * 128              # 512: x columns per sample-group
TGROUPS = [8] * 8                                 # groups per x-tile
CHUNKS = [(0, 4, 32), (4, 3, 24), (7, 1, 8)]
# x-tile -> DMA queue engine. Only the two HWDGE queues (SP=0, Act=1) reach
# the ~360 GB/s aggregate HBM share; adding the Pool SWDGE queue consistently
# REDUCED aggregate bandwidth, so x rides sync/scalar alternating. 1MB tiles
# give 16KB DRAM lines per partition-row pair -> better per-queue rate, and
# only 4 triggers per engine (no queue-credit blocking of Sin).
TQUEUE = [0, 1] * 4
M2 = float(np.float32(1.5 * 2 ** 24))   # round-to-even-integer magic
PI = float(np.pi)
MM_DT = mybir.dt.float16    # PE operand / const dtype
F32 = mybir.dt.float32
AL = mybir.AluOpType
AF = mybir.ActivationFunctionType
AX = mybir.AxisListType
# misc1 (sync queue): wfa 32 | ones 128 | bias 256
MW_WFA, MW_ONES, MW_BIAS, MW1 = 0, 32, 160, 416
# misc2 (scalar queue): Rw 256 | aw 256 | per-chunk 2*8w WC blocks
MW_RW, MW_AW, MW_WC = 0, 256, 512
_WCOFF = [MW_WC]
for (_, _, _w) in CHUNKS:
    _WCOFF.append(_WCOFF[-1] + 2 * 8 * _w)
MW2 = _WCOFF[-1] + (-_WCOFF[-1]) % 64          # pad to 64


def _split_waits(nc, max_waits=1):
    """walrus in this env accepts at most one sync-wait per instruction;
    move extras onto preceding same-engine NoOps."""
    for fn in nc.m.functions:
        for blk in fn.blocks:
            new_list = []
            for inst in blk.instructions:
                si = inst.sync_info
                if si is not None and len(si.on_wait) > max_waits:
                    waits = list(si.on_wait)
                    keep, extra = waits[-max_waits:], waits[:-max_waits]
                    for k, w in enumerate(extra):
                        new_list.append(mybir.InstNoOp(
                            name=f"{inst.name}-ws{k}", engine=inst.engine,
                            ins=[], outs=[],
                            sync_info=mybir.SyncInfo(on_wait=[w], on_update=[])))
                    si.on_wait = keep
                    inst.sync_info = si
                new_list.append(inst)
            blk.instructions = new_list


def _build_nc(bc2):
    """bc2: [2] = b_cls immediates (everything else rides the misc tile)."""
    nc = bass.Bass("TRN2", target_bir_lowering=False)
    # x relayout: xa[p, g*512 + k*128 + ms] = x_core[128 g + ms, 128 k + p]
    xa_d = nc.dram_tensor("xa", [128, BC * NCH], MM_DT, kind="ExternalInput").ap()
    misc1_d = nc.dram_tensor("misc1", [128, MW1], MM_DT, kind="ExternalInput").ap()
    misc2_d = nc.dram_tensor("misc2", [128, MW2], MM_DT, kind="ExternalInput").ap()
    o_d = nc.dram_tensor("o", [128, 2 * NG], F32, kind="ExternalOutput").ap()

    tstart = np.cumsum([0] + TGROUPS)

    with TileContext(nc) as tc:
        with tc.tile_pool(name="wp", bufs=1) as wpool, \
             tc.tile_pool(name="xp", bufs=1) as xpool, \
             tc.tile_pool(name="pp", bufs=1, space="PSUM") as pspool, \
             tc.tile_pool(name="ep", bufs=1) as epool:
            # --- all DMA triggers up front, one const tile per queue,
            # first x tile of each queue triggered BEFORE its const tile ---
            engs = [nc.sync, nc.scalar, nc.gpsimd]
            xts = []

            def xtrig(t):
                xt = xpool.tile([128, TGROUPS[t] * GW], MM_DT,
                                tag=f"xt{t}", name=f"xt{t}")
                engs[TQUEUE[t]].dma_start(
                    xt[:], xa_d[:, tstart[t] * GW:tstart[t + 1] * GW])
                xts.append(xt)

            xtrig(0)
            xtrig(1)
            misc = wpool.tile([128, MW1], MM_DT, name="misc")
            nc.sync.dma_start(misc[:], misc1_d[:])
            misc2 = wpool.tile([128, MW2], MM_DT, name="misc2")
            nc.scalar.dma_start(misc2[:], misc2_d[:])
            ones = misc[0:2, MW_ONES:MW_ONES + 128]
            Rw = misc2[:, MW_RW:MW_RW + 256]
            aw = misc2[:, MW_AW:MW_AW + 256]
            for t in range(2, len(TGROUPS)):
                xtrig(t)

            for ci, (t0, nt, w) in enumerate(CHUNKS):
                W = 8 * w             # chunk PSUM width, col = 8*g + j
                E = pspool.tile([128, W], F32, name=f"E{ci}")
                # phase shift first: E = bs_j (start=True zeroes the region)
                nc.tensor.matmul(E[:, 0:W], ones,
                                 misc[0:2, MW_BIAS:MW_BIAS + W],
                                 start=True, stop=False, skip_group_check=True)
                g0 = 0
                for t in range(t0, t0 + nt):
                    xt = xts[t]
                    for mm in range(TGROUPS[t]):
                        for k in range(NCH):
                            off = mm * GW + k * 128
                            col = 8 * (g0 + mm)
                            nc.tensor.matmul(E[:, col:col + 8],
                                             xt[:, off:off + 128],
                                             misc[:, MW_WFA + 8 * k:MW_WFA + 8 * k + 8],
                                             start=False, stop=(k == NCH - 1),
                                             skip_group_check=True)
                    g0 += TGROUPS[t]

                # ---- epilogue for this chunk ----
                vec, gp = nc.vector, nc.gpsimd
                k2 = epool.tile([128, W], F32, name=f"k2_{ci}")
                r_ = epool.tile([128, W], F32, name=f"r_{ci}")
                s_ = epool.tile([128, W], F32, name=f"s_{ci}")
                t1 = epool.tile([128, W], F32, name=f"t1_{ci}")
                d_ = epool.tile([128, W], F32, name=f"d_{ci}")
                vec.tensor_scalar(k2[:], E[:], M2, M2, AL.add, AL.subtract)
                vec.tensor_sub(r_[:], E[:], k2[:])           # E mod 2 -> [-1, 1]
                nc.scalar.activation(s_[:], r_[:], AF.Sin, scale=PI)
                vec.tensor_mul(t1[:], s_[:], Rw[:, 0:W])
                vec.tensor_add(d_[:], t1[:], aw[:, 0:W])

                # stride-4 views: d4[:, i, :] = d_i for both r, interleaved
                d4 = d_.rearrange("p (u q) -> p q u", q=4)

                def di(i):
                    return d4[:, i, :]                        # [128, 2w] @4

                # products for both reuse halves ([128, 2w], r-interleaved),
                # laid out in k-blocks: Z[:, 2w*k + 2g + r] = z_k^r(group g)
                u_ = epool.tile([128, 2 * w], F32, name=f"u_{ci}")
                Z_ = epool.tile([128, 8 * w], F32, name=f"Z_{ci}")

                def zk(k):
                    return Z_[:, 2 * w * k:2 * w * (k + 1)]

                vec.tensor_mul(u_[:], di(1), di(2))           # d1 d2
                gp.tensor_mul(zk(1), di(0), di(1))            # z1
                vec.tensor_mul(zk(2), di(0), u_[:])           # z2
                gp.tensor_mul(zk(0), u_[:], di(3))            # z0
                gp.tensor_mul(zk(3), zk(2), di(3))            # z3

                # final linear via weighted segmented reduce:
                # out_c(g) = sum_{k,r} 0.5*W_cls[c,k] * z_k^r(g) + b_cls[c]
                O2 = epool.tile([128, 2 * w], F32, name=f"O2_{ci}")
                Zw = epool.tile([128, 2, 8 * w], F32, name=f"Zw_{ci}")
                for c in range(2):
                    wcs = misc2[:, _WCOFF[ci] + c * 8 * w:
                                _WCOFF[ci] + (c + 1) * 8 * w]
                    (vec if c == 0 else gp).tensor_mul(Zw[:, c, :], Z_[:], wcs)
                    red = Zw[:, c, :].rearrange("p (k g r) -> p g k r",
                                                k=4, r=2)    # [p, w, 4, 2]
                    vec.tensor_reduce(O2[:, c * w:(c + 1) * w], red,
                                      AX.XY, AL.add)
                    vec.tensor_scalar_add(O2[:, c * w:(c + 1) * w],
                                          O2[:, c * w:(c + 1) * w], bc2[c])
                off = 2 * tstart[t0]
                nc.sync.dma_start(o_d[:, off:off + 2 * w], O2[:])

    return nc


_NC_CACHE = {}


def _get_nc(consts, split=True):
    (bc2,) = consts
    key = ("nc", split, bc2)
    if key not in _NC_CACHE:
        nc = _build_nc(bc2)
        if split:
            _split_waits(nc)
        _NC_CACHE[key] = nc
    return _NC_CACHE[key]


def _qubit_abc(q_params):
    """Exact (a_i, b_i, c_i) with d_i(theta) = a + b sin(theta) + c cos(theta)."""
    out = np.zeros((NQ, 3), np.float64)
    for i in range(NQ):
        pa, pb, pc = [float(v) for v in q_params[3 * i:3 * i + 3]]

        def rx(t):
            return np.array([[np.cos(t / 2), -1j * np.sin(t / 2)],
                             [-1j * np.sin(t / 2), np.cos(t / 2)]])

        def ry(t):
            return np.array([[np.cos(t / 2), -np.sin(t / 2)],
                             [np.sin(t / 2), np.cos(t / 2)]])

        def rz(t):
            return np.array([[np.exp(-0.5j * t), 0], [0, np.exp(0.5j * t)]])

        H = np.array([[1, 1], [1, -1]]) / np.sqrt(2)
        U = rz(pc) @ ry(pb) @ rx(pa)

        def dfun(theta):
            v = U @ ry(theta) @ H @ np.array([1.0, 0.0])
            pr = np.abs(v) ** 2
            return pr[0] - pr[1]

        d0, dpi, dh = dfun(0.0), dfun(np.pi), dfun(np.pi / 2)
        a = (d0 + dpi) / 2
        c = (d0 - dpi) / 2
        b = dh - a
        out[i] = (a, b, c)
    return out


def _make_consts(b_ctq, q_params, W_cls, b_cls):
    """b_cls immediates + the all-in-one misc const tile (fp16)."""
    abc = _qubit_abc(q_params)
    R8, a8, bs = np.zeros(8), np.zeros(8), np.zeros(8)
    for j in range(8):
        a, b, c_ = abc[j % 4]
        R8[j] = np.hypot(b, c_)
        a8[j] = a
        bs[j] = b_ctq[j] + np.arctan2(c_, b) / np.pi
    bc2 = tuple(float(np.float32(v)) for v in b_cls)

    misc = np.zeros((128, MW1), np.float16)
    misc[:, MW_ONES:MW_ONES + 128] = 1.0
    # bias rows: row0 = fp16 hi, row1 = residual lo (hi+lo == fp32 bs)
    bs_t = np.tile(bs, 256 // 8)
    bhi = bs_t.astype(np.float16)
    misc[0, MW_BIAS:MW_BIAS + 256] = bhi
    misc[1, MW_BIAS:MW_BIAS + 256] = (bs_t - bhi.astype(np.float64)).astype(np.float16)
    misc2 = np.zeros((128, MW2), np.float16)
    misc2[:, MW_RW:MW_RW + 256] = np.tile(R8, 256 // 8).astype(np.float16)
    misc2[:, MW_AW:MW_AW + 256] = np.tile(a8, 256 // 8).astype(np.float16)
    wp = 0.5 * np.asarray(W_cls, np.float64)      # mean over r folded in
    for ci, (_, _, w) in enumerate(CHUNKS):
        for c in range(2):
            for k in range(4):
                lo = _WCOFF[ci] + c * 8 * w + k * 2 * w
                misc2[:, lo:lo + 2 * w] = np.float16(wp[c, k])
    return (bc2,), misc, misc2


def make_in_maps(x, W_ctq, b_ctq, q_params, W_cls, b_cls):
    consts, misc, misc2 = _make_consts(np.asarray(b_ctq, np.float32),
                                       np.asarray(q_params, np.float32),
                                       np.asarray(W_cls, np.float32),
                                       np.asarray(b_cls, np.float32))
    wt = np.asarray(W_ctq, np.float32).T                        # [512, 8]
    misc[:, MW_WFA:MW_WFA + 32] = \
        wt.reshape(NCH, 128, 8).transpose(1, 0, 2).reshape(128, 32)
    misc = np.ascontiguousarray(misc)
    x = np.asarray(x, np.float32)
    in_maps = []
    for c in range(NCORES):
        xs = x[c * BC:(c + 1) * BC]                             # [8192, 512]
        # relayout: [p, g*512 + k*128 + ms] = xs[128 g + ms, 128 k + p]
        xa = np.ascontiguousarray(
            xs.reshape(NG, 128, NCH, 128).transpose(3, 0, 2, 1)
              .reshape(128, BC * NCH)).astype(np.float16)
        in_maps.append({"xa": xa, "misc1": misc, "misc2": misc2})
    return in_maps, consts


def assemble_output(results):
    tstart = np.cumsum([0] + TGROUPS)
    out = np.empty((B, 2), np.float32)
    for core in range(NCORES):
        o = results[core]["o"]                                   # [128, 2*NG]
        for (t0, nt, w) in CHUNKS:
            gb, off = tstart[t0], 2 * tstart[t0]
            for c in range(2):
                # o[p, off + c*w + g] = out_c(sample 128 (gb+g) + p)
                out[core * BC + 128 * gb:core * BC + 128 * (gb + w), c] = \
                    o[:, off + c * w:off + (c + 1) * w].T.reshape(-1)
    return out


def kernel(x, W_ctq, b_ctq, q_params, W_cls, b_cls):
    in_maps, consts = make_in_maps(x, W_ctq, b_ctq, q_params, W_cls, b_cls)
    nc = _get_nc(consts)
    res = bass_utils.run_bass_kernel_spmd(nc, in_maps, core_ids=list(range(NCORES)))
    return assemble_output(res.results)
